# revision 6
# baseline (speedup 1.0000x reference)
"""Trainium2 Bass kernel for nn_Conv_8443905704574.

Reference semantics: 7x7 cross-correlation (stride 1, zero pad 3) applied to
the LAST input channel only; the single-channel result is broadcast to all 3
output channels.

Device algorithm: banded-Toeplitz matmul conv in bf16 using 32x32 PE-array
tiling. The 128x128 PE array is addressed as 16 independent 32x32 tiles
(tile_position=(32*ki, 32*mj)); 16 matmuls issue back-to-back and execute
concurrently on the sub-arrays (~436 ns per 16-MM slot at N=512, vs 216 ns
for ONE full-128 matmul). Each tile convolves a 32-row window of the image
producing 26 valid output rows: the stationary is a [32,32] band matrix
(T[k,m] = K[k-m,dj]) per kernel column dj, the moving operand a W-shifted
[32,512] slice; 7 taps accumulate in fp32 PSUM. One "round" = 16 row-groups
x 2 W-chunks = 14 concurrent-16 slots; 5 rounds cover a core's 2 images.

DMA: every transfer spans all 128 SBUF partitions (the HWDGE sprays
descriptors across all 16 SDMA engines only for 128-partition transfers;
partial-partition stores collapse onto 2 engines at ~45 GB/s). Host packs
x[128, 5, 4, 1030] bf16 (partition 32*ki+q of (round r, mj) holds padded
image row 26*(16r+4mj+ki)+q) and unpacks y[128, 5, 4, 1024] bf16. PSUM banks
are drained by Scalar (ki 0-1) and Vector (ki 2-3) engines in parallel,
casting fp32->bf16.

Sharding: pure data parallel - 2 images per core across 8 cores.
"""

import numpy as np
import ml_dtypes

import concourse.bacc as bacc
import concourse.mybir as mybir
import concourse.tile as tile
from concourse.bass_utils import run_bass_kernel_spmd

B, C, H, W = 16, 3, 1024, 1024
KS = 7
PAD = KS // 2
NCORES = 8
PER = B // NCORES            # images per core
GR = 32 - (KS - 1)           # 26 valid output rows per 32-row tile window
NGI = (H + GR - 1) // GR     # 40 row-groups per image
NG = PER * NGI               # 80 row-groups per core
ROUNDS = NG // 16            # 5 rounds of 16 concurrent tiles
XW = W + 2 * PAD             # host-padded input width (1030)
HP = GR * (NGI - 1) + 32     # host-padded input height (1046)

f32 = mybir.dt.float32
bf16 = mybir.dt.bfloat16

_CACHE = {}
LAST_RESULTS = None


def _build_bass():
    nc = bacc.Bacc("TRN2", target_bir_lowering=False, debug=False)
    x = nc.dram_tensor("x", [128, ROUNDS, 4, XW], bf16, kind="ExternalInput")
    tmat = nc.dram_tensor("tmat", [128, KS * 32], bf16, kind="ExternalInput")
    y = nc.dram_tensor("y", [128, ROUNDS, 4, W], bf16, kind="ExternalOutput")

    with tile.TileContext(nc) as tc:
        with (
            tc.tile_pool(name="xp", bufs=ROUNDS) as xpool,
            tc.tile_pool(name="tp", bufs=1) as tpool,
            tc.tile_pool(name="op", bufs=2) as opool,
            tc.tile_pool(name="pp", bufs=1, space="PSUM") as ppool,
            tc.tile_pool(name="wp", bufs=1) as wzpool,
        ):
            # 8 PSUM banks: (ki, chunk) -> one [128,512] bank holding the
            # 4 mj tiles' outputs stacked along partitions.
            ps = [
                [
                    ppool.tile([128, 512], f32, name=f"ps{ki}{c}", tag=f"ps{ki}{c}")
                    for c in range(2)
                ]
                for ki in range(4)
            ]

            # Stationaries first (tiny, gates the first real matmul), then
            # all input rounds up front — bufs=ROUNDS, so no reuse hazard
            # and the sync DGE FIFO never blocks on a compute dependency.
            ts = tpool.tile([128, KS * 32], bf16, name="ts")
            nc.sync.dma_start(ts[:], tmat[:])

            xgs = []
            for r in range(ROUNDS):
                xg = xpool.tile([128, 4 * XW], bf16, name=f"xg", tag="xg")
                xgs.append(xg)
                nc.sync.dma_start(xg[:], x[:, r, :, :])

            # PE warm-up: zero matmuls release the HAM clock gate so real
            # matmuls run at 2.4 GHz. Writes land in ps banks and are
            # overwritten by the first start=True tap.
            wz = wzpool.tile([128, 128 + 512], bf16, name="wz")
            nc.vector.memset(wz[:], 0.0)
            for i in range(18):
                nc.tensor.matmul(
                    ps[i % 2][0][:],
                    wz[:, 0:128],
                    wz[:, 128 : 128 + 512],
                    start=True,
                    stop=True,
                )

            for r in range(ROUNDS):
                xg = xgs[r]
                # dj outer, chunk inner: consecutive 16-MM slots reuse the
                # same stationary per tile (chance for LDWEIGHTS elision).
                for dj in range(KS):
                    for c in range(2):
                        for ki in range(4):
                            for mj in range(4):
                                nc.tensor.matmul(
                                    ps[ki][c][32 * mj : 32 * mj + 32, :],
                                    ts[32 * ki : 32 * ki + 32, dj * 32 : dj * 32 + 32],
                                    xg[
                                        32 * ki : 32 * ki + 32,
                                        mj * XW + c * 512 + dj : mj * XW + c * 512 + dj + 512,
                                    ],
                                    start=(dj == 0),
                                    stop=(dj == KS - 1),
                                    tile_position=(32 * ki, 32 * mj),
                                )
                # Drain per ki and store immediately: the 4 small stores
                # overlap the next round's matmuls, shrinking the tail.
                for ki in range(4):
                    otk = opool.tile([128, W], bf16, name=f"ot{ki}", tag=f"ot{ki}")
                    for c in range(2):
                        dst = otk[:, c * 512 : c * 512 + 512]
                        if ki < 2:
                            nc.scalar.copy(dst, ps[ki][c][:])
                        else:
                            nc.vector.tensor_copy(dst, ps[ki][c][:])
                    nc.sync.dma_start(y[:, r, ki, :], otk[:])
    nc.compile()
    return nc


def _toeplitz(kmat: np.ndarray) -> np.ndarray:
    """[128, KS*32] bf16: four identical [32, KS*32] stationary band-matrix
    strips (one per PE row-group). T[k, dj*32+m] = K[k-m, dj] for k-m in
    [0, KS)."""
    k_idx = np.arange(32)[:, None]
    m_idx = np.arange(32)[None, :]
    di = k_idx - m_idx
    mask = (di >= 0) & (di < KS)
    dic = np.clip(di, 0, KS - 1)
    t = np.zeros((32, KS, 32), dtype=np.float32)
    for dj in range(KS):
        t[:, dj, :] = np.where(mask, kmat[dic, dj], 0.0)
    t = t.reshape(32, KS * 32)
    return np.tile(t, (4, 1)).astype(ml_dtypes.bfloat16)


def _shard_inputs(image: np.ndarray, kmat: np.ndarray):
    tmat = _toeplitz(kmat)
    xb = image[:, C - 1].astype(ml_dtypes.bfloat16)  # [B, H, W]
    pad = np.zeros((B, HP, XW), dtype=ml_dtypes.bfloat16)
    pad[:, PAD : PAD + H, PAD : PAD + W] = xb

    p = np.arange(128)
    ki = (p >> 5)[:, None, None]                      # [128,1,1]
    q = (p & 31)[:, None, None]
    r = np.arange(ROUNDS)[None, :, None]              # [1,R,1]
    mj = np.arange(4)[None, None, :]                  # [1,1,4]
    g = 16 * r + 4 * mj + ki                          # [128,R,4] core-group id
    img_loc = g // NGI
    row = GR * (g % NGI) + q                          # [128,R,4]

    in_maps = []
    for i in range(NCORES):
        xi = pad[2 * i + img_loc, row, :]             # [128,R,4,XW]
        in_maps.append({"x": np.ascontiguousarray(xi), "tmat": tmat})
    return in_maps


def _unpack_output(results) -> np.ndarray:
    y = np.empty((B, H, W), dtype=np.float32)
    for i in range(NCORES):
        arr = np.asarray(results[i]["y"]).astype(np.float32)  # [128,R,4,W]
        for r in range(ROUNDS):
            for mj in range(4):
                for ki in range(4):
                    g = 16 * r + 4 * mj + ki
                    img = PER * i + g // NGI
                    r0 = GR * (g % NGI)
                    nv = min(GR, H - r0)
                    y[img, r0 : r0 + nv] = arr[32 * mj : 32 * mj + nv, r, ki]
    return y


def kernel(**inputs):
    global LAST_RESULTS
    image = np.asarray(inputs["image"], dtype=np.float32)
    kmat = np.asarray(inputs["kernel"], dtype=np.float32)
    assert image.shape == (B, C, H, W), image.shape

    if "nc" not in _CACHE:
        _CACHE["nc"] = _build_bass()
    nc = _CACHE["nc"]

    in_maps = _shard_inputs(image, kmat)
    res = run_bass_kernel_spmd(nc, in_maps, list(range(NCORES)))
    LAST_RESULTS = res

    y = _unpack_output(res.results)
    return np.broadcast_to(y[:, None], (B, C, H, W))


# revision 7
# speedup vs baseline: 1.0837x; 1.0837x over previous
"""Trainium2 Bass kernel for nn_Conv_8443905704574.

Reference semantics: 7x7 cross-correlation (stride 1, zero pad 3) applied to
the LAST input channel only; the single-channel result is broadcast to all 3
output channels.

Device algorithm: banded-Toeplitz matmul conv in bf16 using 32x32 PE-array
tiling. The 128x128 PE array is addressed as 16 independent 32x32 tiles
(tile_position=(32*ki, 32*mj)); 16 matmuls issue back-to-back and execute
concurrently on the sub-arrays (~436 ns per 16-MM slot at N=512, vs 216 ns
for ONE full-128 matmul). Each tile convolves a 32-row window of the image
producing 26 valid output rows: the stationary is a [32,32] band matrix
(T[k,m] = K[k-m,dj]) per kernel column dj, the moving operand a W-shifted
[32,512] slice; 7 taps accumulate in fp32 PSUM. One "round" = 16 row-groups
x 2 W-chunks = 14 concurrent-16 slots; 5 rounds cover a core's 2 images.

DMA: every transfer spans all 128 SBUF partitions (the HWDGE sprays
descriptors across all 16 SDMA engines only for 128-partition transfers;
partial-partition stores collapse onto 2 engines at ~45 GB/s). Host packs
x[128, 5, 4, 1030] bf16 (partition 32*ki+q of (round r, mj) holds padded
image row 26*(16r+4mj+ki)+q) and unpacks y[128, 5, 4, 1024] bf16. PSUM banks
are drained by Scalar (ki 0-1) and Vector (ki 2-3) engines in parallel,
casting fp32->bf16.

Sharding: pure data parallel - 2 images per core across 8 cores.
"""

import numpy as np
import ml_dtypes

import concourse.bacc as bacc
import concourse.mybir as mybir
import concourse.tile as tile
from concourse.bass_utils import run_bass_kernel_spmd

B, C, H, W = 16, 3, 1024, 1024
KS = 7
PAD = KS // 2
NCORES = 8
PER = B // NCORES            # images per core
GR = 32 - (KS - 1)           # 26 valid output rows per 32-row tile window
NGI = (H + GR - 1) // GR     # 40 row-groups per image
NG = PER * NGI               # 80 row-groups per core
ROUNDS = NG // 16            # 5 rounds of 16 concurrent tiles
XW = W + 2 * PAD             # host-padded input width (1030)
HP = GR * (NGI - 1) + 32     # host-padded input height (1046)

f32 = mybir.dt.float32
bf16 = mybir.dt.bfloat16

_CACHE = {}
LAST_RESULTS = None


def _build_bass():
    nc = bacc.Bacc("TRN2", target_bir_lowering=False, debug=False)
    x = nc.dram_tensor("x", [128, ROUNDS, 4, XW], bf16, kind="ExternalInput")
    tmat = nc.dram_tensor("tmat", [128, KS * 32], bf16, kind="ExternalInput")
    y = nc.dram_tensor("y", [128, ROUNDS, 4, W], bf16, kind="ExternalOutput")

    with tile.TileContext(nc) as tc:
        with (
            tc.tile_pool(name="xp", bufs=ROUNDS) as xpool,
            tc.tile_pool(name="tp", bufs=1) as tpool,
            tc.tile_pool(name="op", bufs=2) as opool,
            tc.tile_pool(name="pp", bufs=1, space="PSUM") as ppool,
            tc.tile_pool(name="wp", bufs=1) as wzpool,
        ):
            # 8 PSUM banks: (ki, chunk) -> one [128,512] bank holding the
            # 4 mj tiles' outputs stacked along partitions.
            ps = [
                [
                    ppool.tile([128, 512], f32, name=f"ps{ki}{c}", tag=f"ps{ki}{c}")
                    for c in range(2)
                ]
                for ki in range(4)
            ]

            # Stationaries first (tiny, gates the first real matmul), then
            # all input rounds up front — bufs=ROUNDS, so no reuse hazard
            # and the sync DGE FIFO never blocks on a compute dependency.
            ts = tpool.tile([128, KS * 32], bf16, name="ts")
            nc.sync.dma_start(ts[:], tmat[:])

            xgs = []
            for r in range(ROUNDS):
                xg = xpool.tile([128, 4 * XW], bf16, name=f"xg", tag="xg")
                xgs.append(xg)
                nc.sync.dma_start(xg[:], x[:, r, :, :])

            # PE warm-up: zero matmuls release the HAM clock gate so real
            # matmuls run at 2.4 GHz. Writes land in ps banks and are
            # overwritten by the first start=True tap.
            wz = wzpool.tile([128, 128 + 512], bf16, name="wz")
            nc.vector.memset(wz[:], 0.0)
            for i in range(18):
                nc.tensor.matmul(
                    ps[i % 2][0][:],
                    wz[:, 0:128],
                    wz[:, 128 : 128 + 512],
                    start=True,
                    stop=True,
                )

            for r in range(ROUNDS):
                xg = xgs[r]
                for c in range(2):
                    for dj in range(KS):
                        for ki in range(4):
                            for mj in range(4):
                                nc.tensor.matmul(
                                    ps[ki][c][32 * mj : 32 * mj + 32, :],
                                    ts[32 * ki : 32 * ki + 32, dj * 32 : dj * 32 + 32],
                                    xg[
                                        32 * ki : 32 * ki + 32,
                                        mj * XW + c * 512 + dj : mj * XW + c * 512 + dj + 512,
                                    ],
                                    start=(dj == 0),
                                    stop=(dj == KS - 1),
                                    tile_position=(32 * ki, 32 * mj),
                                )
                ot = opool.tile([128, 4 * W], bf16, name="ot", tag="ot")
                for ki in range(4):
                    for c in range(2):
                        dst = ot[:, ki * W + c * 512 : ki * W + c * 512 + 512]
                        if ki < 2:
                            nc.scalar.copy(dst, ps[ki][c][:])
                        else:
                            nc.vector.tensor_copy(dst, ps[ki][c][:])
                nc.sync.dma_start(y[:, r, :, :], ot[:])
    nc.compile()
    return nc


def _toeplitz(kmat: np.ndarray) -> np.ndarray:
    """[128, KS*32] bf16: four identical [32, KS*32] stationary band-matrix
    strips (one per PE row-group). T[k, dj*32+m] = K[k-m, dj] for k-m in
    [0, KS)."""
    k_idx = np.arange(32)[:, None]
    m_idx = np.arange(32)[None, :]
    di = k_idx - m_idx
    mask = (di >= 0) & (di < KS)
    dic = np.clip(di, 0, KS - 1)
    t = np.zeros((32, KS, 32), dtype=np.float32)
    for dj in range(KS):
        t[:, dj, :] = np.where(mask, kmat[dic, dj], 0.0)
    t = t.reshape(32, KS * 32)
    return np.tile(t, (4, 1)).astype(ml_dtypes.bfloat16)


def _shard_inputs(image: np.ndarray, kmat: np.ndarray):
    tmat = _toeplitz(kmat)
    xb = image[:, C - 1].astype(ml_dtypes.bfloat16)  # [B, H, W]
    pad = np.zeros((B, HP, XW), dtype=ml_dtypes.bfloat16)
    pad[:, PAD : PAD + H, PAD : PAD + W] = xb

    p = np.arange(128)
    ki = (p >> 5)[:, None, None]                      # [128,1,1]
    q = (p & 31)[:, None, None]
    r = np.arange(ROUNDS)[None, :, None]              # [1,R,1]
    mj = np.arange(4)[None, None, :]                  # [1,1,4]
    g = 16 * r + 4 * mj + ki                          # [128,R,4] core-group id
    img_loc = g // NGI
    row = GR * (g % NGI) + q                          # [128,R,4]

    in_maps = []
    for i in range(NCORES):
        xi = pad[2 * i + img_loc, row, :]             # [128,R,4,XW]
        in_maps.append({"x": np.ascontiguousarray(xi), "tmat": tmat})
    return in_maps


def _unpack_output(results) -> np.ndarray:
    y = np.empty((B, H, W), dtype=np.float32)
    for i in range(NCORES):
        arr = np.asarray(results[i]["y"]).astype(np.float32)  # [128,R,4,W]
        for r in range(ROUNDS):
            for mj in range(4):
                for ki in range(4):
                    g = 16 * r + 4 * mj + ki
                    img = PER * i + g // NGI
                    r0 = GR * (g % NGI)
                    nv = min(GR, H - r0)
                    y[img, r0 : r0 + nv] = arr[32 * mj : 32 * mj + nv, r, ki]
    return y


def kernel(**inputs):
    global LAST_RESULTS
    image = np.asarray(inputs["image"], dtype=np.float32)
    kmat = np.asarray(inputs["kernel"], dtype=np.float32)
    assert image.shape == (B, C, H, W), image.shape

    if "nc" not in _CACHE:
        _CACHE["nc"] = _build_bass()
    nc = _CACHE["nc"]

    in_maps = _shard_inputs(image, kmat)
    res = run_bass_kernel_spmd(nc, in_maps, list(range(NCORES)))
    LAST_RESULTS = res

    y = _unpack_output(res.results)
    return np.broadcast_to(y[:, None], (B, C, H, W))


# revision 9
# speedup vs baseline: 1.1198x; 1.0333x over previous
"""Trainium2 Bass kernel for nn_Conv_8443905704574.

Reference semantics: 7x7 cross-correlation (stride 1, zero pad 3) applied to
the LAST input channel only; the single-channel result is broadcast to all 3
output channels.

Device algorithm: banded-Toeplitz matmul conv in bf16 using 32x32 PE-array
tiling. The 128x128 PE array is addressed as 16 independent 32x32 tiles
(tile_position=(32*ki, 32*mj)); 16 matmuls issue back-to-back and execute
concurrently on the sub-arrays (~436 ns per 16-MM slot at N=512, vs 216 ns
for ONE full-128 matmul). Each tile convolves a 32-row window of the image
producing 26 valid output rows: the stationary is a [32,32] band matrix
(T[k,m] = K[k-m,dj]) per kernel column dj, the moving operand a W-shifted
[32,512] slice; 7 taps accumulate in fp32 PSUM. One "round" = 16 row-groups
x 2 W-chunks = 14 concurrent-16 slots; 5 rounds cover a core's 2 images.

DMA: every transfer spans all 128 SBUF partitions (the HWDGE sprays
descriptors across all 16 SDMA engines only for 128-partition transfers;
partial-partition stores collapse onto 2 engines at ~45 GB/s). Host packs
x[128, 5, 4, 1030] bf16 (partition 32*ki+q of (round r, mj) holds padded
image row 26*(16r+4mj+ki)+q) and unpacks y[128, 5, 4, 1024] bf16. PSUM banks
are drained by Scalar (ki 0-1) and Vector (ki 2-3) engines in parallel,
casting fp32->bf16.

Sharding: pure data parallel - 2 images per core across 8 cores.
"""

import numpy as np
import ml_dtypes

import concourse.bacc as bacc
import concourse.mybir as mybir
import concourse.tile as tile
from concourse.bass_utils import run_bass_kernel_spmd

B, C, H, W = 16, 3, 1024, 1024
KS = 7
PAD = KS // 2
NCORES = 8
PER = B // NCORES            # images per core
GR = 32 - (KS - 1)           # 26 valid output rows per 32-row tile window
NGI = (H + GR - 1) // GR     # 40 row-groups per image
NG = PER * NGI               # 80 row-groups per core
ROUNDS = NG // 16            # 5 rounds of 16 concurrent tiles
XW = W + 2 * PAD             # host-padded input width (1030)
HP = GR * (NGI - 1) + 32     # host-padded input height (1046)

f32 = mybir.dt.float32
bf16 = mybir.dt.bfloat16

_CACHE = {}
LAST_RESULTS = None


def _build_bass():
    nc = bacc.Bacc("TRN2", target_bir_lowering=False, debug=False)
    x = nc.dram_tensor("x", [128, ROUNDS, 4, XW], bf16, kind="ExternalInput")
    tmat = nc.dram_tensor("tmat", [128, KS * 32], bf16, kind="ExternalInput")
    y = nc.dram_tensor("y", [128, ROUNDS, 4, W], bf16, kind="ExternalOutput")

    with tile.TileContext(nc) as tc:
        with (
            tc.tile_pool(name="xp", bufs=ROUNDS) as xpool,
            tc.tile_pool(name="tp", bufs=1) as tpool,
            tc.tile_pool(name="op", bufs=2) as opool,
            tc.tile_pool(name="pp", bufs=1, space="PSUM") as ppool,
            tc.tile_pool(name="wp", bufs=1) as wzpool,
        ):
            # 8 PSUM banks: (ki, chunk) -> one [128,512] bank holding the
            # 4 mj tiles' outputs stacked along partitions.
            ps = [
                [
                    ppool.tile([128, 512], f32, name=f"ps{ki}{c}", tag=f"ps{ki}{c}")
                    for c in range(2)
                ]
                for ki in range(4)
            ]

            # Stationaries first (tiny, gates the first real matmul), then
            # all input rounds up front — bufs=ROUNDS, so no reuse hazard
            # and the sync DGE FIFO never blocks on a compute dependency.
            ts = tpool.tile([128, KS * 32], bf16, name="ts")
            nc.sync.dma_start(ts[:], tmat[:])

            xgs = []
            for r in range(ROUNDS):
                xg = xpool.tile([128, 4 * XW], bf16, name=f"xg", tag="xg")
                xgs.append(xg)
                nc.sync.dma_start(xg[:], x[:, r, :, :])

            # PE warm-up: zero matmuls release the HAM clock gate so real
            # matmuls run at 2.4 GHz. Writes land in ps banks and are
            # overwritten by the first start=True tap.
            wz = wzpool.tile([128, 128 + 512], bf16, name="wz")
            nc.vector.memset(wz[:], 0.0)
            for i in range(12):
                nc.tensor.matmul(
                    ps[i % 2][0][:],
                    wz[:, 0:128],
                    wz[:, 128 : 128 + 512],
                    start=True,
                    stop=True,
                )

            for r in range(ROUNDS):
                xg = xgs[r]
                for c in range(2):
                    for dj in range(KS):
                        for ki in range(4):
                            for mj in range(4):
                                nc.tensor.matmul(
                                    ps[ki][c][32 * mj : 32 * mj + 32, :],
                                    ts[32 * ki : 32 * ki + 32, dj * 32 : dj * 32 + 32],
                                    xg[
                                        32 * ki : 32 * ki + 32,
                                        mj * XW + c * 512 + dj : mj * XW + c * 512 + dj + 512,
                                    ],
                                    start=(dj == 0),
                                    stop=(dj == KS - 1),
                                    tile_position=(32 * ki, 32 * mj),
                                )
                if r < ROUNDS - 1:
                    ot = opool.tile([128, 4 * W], bf16, name="ot", tag="ot")
                    for ki in range(4):
                        for c in range(2):
                            dst = ot[:, ki * W + c * 512 : ki * W + c * 512 + 512]
                            if ki < 2:
                                nc.scalar.copy(dst, ps[ki][c][:])
                            else:
                                nc.vector.tensor_copy(dst, ps[ki][c][:])
                    nc.sync.dma_start(y[:, r, :, :], ot[:])
                else:
                    # Last round: store per ki as soon as its two banks are
                    # drained, so the final transfers overlap the drain.
                    for ki in range(4):
                        otk = opool.tile(
                            [128, W], bf16, name=f"otl{ki}", tag=f"otl{ki}"
                        )
                        for c in range(2):
                            dst = otk[:, c * 512 : c * 512 + 512]
                            if ki < 2:
                                nc.scalar.copy(dst, ps[ki][c][:])
                            else:
                                nc.vector.tensor_copy(dst, ps[ki][c][:])
                        nc.sync.dma_start(y[:, r, ki, :], otk[:])
    nc.compile()
    return nc


def _toeplitz(kmat: np.ndarray) -> np.ndarray:
    """[128, KS*32] bf16: four identical [32, KS*32] stationary band-matrix
    strips (one per PE row-group). T[k, dj*32+m] = K[k-m, dj] for k-m in
    [0, KS)."""
    k_idx = np.arange(32)[:, None]
    m_idx = np.arange(32)[None, :]
    di = k_idx - m_idx
    mask = (di >= 0) & (di < KS)
    dic = np.clip(di, 0, KS - 1)
    t = np.zeros((32, KS, 32), dtype=np.float32)
    for dj in range(KS):
        t[:, dj, :] = np.where(mask, kmat[dic, dj], 0.0)
    t = t.reshape(32, KS * 32)
    return np.tile(t, (4, 1)).astype(ml_dtypes.bfloat16)


def _shard_inputs(image: np.ndarray, kmat: np.ndarray):
    tmat = _toeplitz(kmat)
    xb = image[:, C - 1].astype(ml_dtypes.bfloat16)  # [B, H, W]
    pad = np.zeros((B, HP, XW), dtype=ml_dtypes.bfloat16)
    pad[:, PAD : PAD + H, PAD : PAD + W] = xb

    p = np.arange(128)
    ki = (p >> 5)[:, None, None]                      # [128,1,1]
    q = (p & 31)[:, None, None]
    r = np.arange(ROUNDS)[None, :, None]              # [1,R,1]
    mj = np.arange(4)[None, None, :]                  # [1,1,4]
    g = 16 * r + 4 * mj + ki                          # [128,R,4] core-group id
    img_loc = g // NGI
    row = GR * (g % NGI) + q                          # [128,R,4]

    in_maps = []
    for i in range(NCORES):
        xi = pad[2 * i + img_loc, row, :]             # [128,R,4,XW]
        in_maps.append({"x": np.ascontiguousarray(xi), "tmat": tmat})
    return in_maps


def _unpack_output(results) -> np.ndarray:
    y = np.empty((B, H, W), dtype=np.float32)
    for i in range(NCORES):
        arr = np.asarray(results[i]["y"]).astype(np.float32)  # [128,R,4,W]
        for r in range(ROUNDS):
            for mj in range(4):
                for ki in range(4):
                    g = 16 * r + 4 * mj + ki
                    img = PER * i + g // NGI
                    r0 = GR * (g % NGI)
                    nv = min(GR, H - r0)
                    y[img, r0 : r0 + nv] = arr[32 * mj : 32 * mj + nv, r, ki]
    return y


def kernel(**inputs):
    global LAST_RESULTS
    image = np.asarray(inputs["image"], dtype=np.float32)
    kmat = np.asarray(inputs["kernel"], dtype=np.float32)
    assert image.shape == (B, C, H, W), image.shape

    if "nc" not in _CACHE:
        _CACHE["nc"] = _build_bass()
    nc = _CACHE["nc"]

    in_maps = _shard_inputs(image, kmat)
    res = run_bass_kernel_spmd(nc, in_maps, list(range(NCORES)))
    LAST_RESULTS = res

    y = _unpack_output(res.results)
    return np.broadcast_to(y[:, None], (B, C, H, W))


# revision 10
# speedup vs baseline: 1.2715x; 1.1355x over previous
"""Trainium2 Bass kernel for nn_Conv_8443905704574.

Reference semantics: 7x7 cross-correlation (stride 1, zero pad 3) applied to
the LAST input channel only; the single-channel result is broadcast to all 3
output channels.

Device algorithm: banded-Toeplitz matmul conv in bf16 using 64x64 PE-array
tiling. The 128x128 PE array is addressed as four 64x64 tiles
(tile_position=(64*ki, 64*mj)); the 4 matmuls of a tap-slot issue
back-to-back and execute concurrently on the quadrants (measured ~216 ns per
4-MM slot at N=512 vs 547 ns for a 16-MM 32x32 slot — fewer instructions and
a 1-wave XBUS schedule). Each tile convolves a 64-row window of the image
producing 58 valid output rows: the stationary is a [64,64] band matrix
(T[k,m] = K[k-m,dj]) per kernel column dj, the moving operand a W-shifted
[64,512] slice; 7 taps accumulate in fp32 PSUM. One round = 4 row-groups x
2 W-chunks = 14 slots; 9 rounds cover a core's 2 images.

DMA: every transfer spans all 128 SBUF partitions (the HWDGE sprays
descriptors across all 16 SDMA engines only for 128-partition transfers;
partial-partition stores collapse onto 2 engines at ~45 GB/s). Host packs
x[128, 9, 2, 1030] bf16 (partition 64*ki+q of (round r, mj) holds padded
image row 58*(4r+2mj+ki)+q) and unpacks y[128, 9, 2, 1024] bf16. PSUM banks
are drained by Scalar (ki=0) and Vector (ki=1) engines in parallel, casting
fp32->bf16; the last round stores per-ki so the final transfers overlap the
drain.

Sharding: pure data parallel - 2 images per core across 8 cores.
"""

import numpy as np
import ml_dtypes

import concourse.bacc as bacc
import concourse.mybir as mybir
import concourse.tile as tile
from concourse.bass_utils import run_bass_kernel_spmd

B, C, H, W = 16, 3, 1024, 1024
KS = 7
PAD = KS // 2
NCORES = 8
PER = B // NCORES            # images per core
GR = 64 - (KS - 1)           # 58 valid output rows per 64-row tile window
NGI = (H + GR - 1) // GR     # 18 row-groups per image
NG = PER * NGI               # 36 row-groups per core
ROUNDS = NG // 4             # 9 rounds of 4 concurrent tiles
XW = W + 2 * PAD             # host-padded input width (1030)
HP = GR * (NGI - 1) + 64     # host-padded input height (1050)

f32 = mybir.dt.float32
bf16 = mybir.dt.bfloat16

_CACHE = {}
LAST_RESULTS = None


def _build_bass():
    nc = bacc.Bacc("TRN2", target_bir_lowering=False, debug=False)
    x = nc.dram_tensor("x", [128, ROUNDS, 2, XW], bf16, kind="ExternalInput")
    tmat = nc.dram_tensor("tmat", [128, KS * 64], bf16, kind="ExternalInput")
    y = nc.dram_tensor("y", [128, ROUNDS, 2, W], bf16, kind="ExternalOutput")

    with tile.TileContext(nc) as tc:
        with (
            tc.tile_pool(name="xp", bufs=ROUNDS) as xpool,
            tc.tile_pool(name="tp", bufs=1) as tpool,
            tc.tile_pool(name="op", bufs=2) as opool,
            tc.tile_pool(name="pp", bufs=1, space="PSUM") as ppool,
            tc.tile_pool(name="wp", bufs=1) as wzpool,
        ):
            # 4 PSUM banks: (ki, chunk) -> one [128,512] bank holding the
            # 2 mj tiles' outputs stacked along partitions; +2 warmup banks.
            ps = [
                [
                    ppool.tile([128, 512], f32, name=f"ps{ki}{c}", tag=f"ps{ki}{c}")
                    for c in range(2)
                ]
                for ki in range(2)
            ]
            pz = [
                ppool.tile([128, 512], f32, name=f"pz{i}", tag=f"pz{i}")
                for i in range(2)
            ]

            # Stationaries first (tiny, gates the first real matmul), then
            # all input rounds up front - bufs=ROUNDS, so no reuse hazard
            # and the sync DGE FIFO never blocks on a compute dependency.
            ts = tpool.tile([128, KS * 64], bf16, name="ts")
            nc.sync.dma_start(ts[:], tmat[:])

            xgs = []
            for r in range(ROUNDS):
                xg = xpool.tile([128, 2 * XW], bf16, name=f"xg", tag="xg")
                xgs.append(xg)
                nc.sync.dma_start(xg[:], x[:, r, :, :])

            # PE warm-up: zero matmuls release the HAM clock gate so real
            # matmuls run at 2.4 GHz.
            wz = wzpool.tile([128, 128 + 512], bf16, name="wz")
            nc.vector.memset(wz[:], 0.0)
            for i in range(12):
                nc.tensor.matmul(
                    pz[i % 2][:],
                    wz[:, 0:128],
                    wz[:, 128 : 128 + 512],
                    start=True,
                    stop=True,
                )

            for r in range(ROUNDS):
                xg = xgs[r]
                for c in range(2):
                    for dj in range(KS):
                        for ki in range(2):
                            for mj in range(2):
                                nc.tensor.matmul(
                                    ps[ki][c][64 * mj : 64 * mj + 64, :],
                                    ts[64 * ki : 64 * ki + 64, dj * 64 : dj * 64 + 64],
                                    xg[
                                        64 * ki : 64 * ki + 64,
                                        mj * XW + c * 512 + dj : mj * XW + c * 512 + dj + 512,
                                    ],
                                    start=(dj == 0),
                                    stop=(dj == KS - 1),
                                    tile_position=(64 * ki, 64 * mj),
                                )
                if r < ROUNDS - 1:
                    ot = opool.tile([128, 2 * W], bf16, name="ot", tag="ot")
                    for ki in range(2):
                        for c in range(2):
                            dst = ot[:, ki * W + c * 512 : ki * W + c * 512 + 512]
                            if ki == 0:
                                nc.scalar.copy(dst, ps[ki][c][:])
                            else:
                                nc.vector.tensor_copy(dst, ps[ki][c][:])
                    nc.sync.dma_start(y[:, r, :, :], ot[:])
                else:
                    # Last round: store per ki as soon as its two banks are
                    # drained, so the final transfers overlap the drain.
                    for ki in range(2):
                        otk = opool.tile(
                            [128, W], bf16, name=f"otl{ki}", tag=f"otl{ki}"
                        )
                        for c in range(2):
                            dst = otk[:, c * 512 : c * 512 + 512]
                            if ki == 0:
                                nc.scalar.copy(dst, ps[ki][c][:])
                            else:
                                nc.vector.tensor_copy(dst, ps[ki][c][:])
                        nc.sync.dma_start(y[:, r, ki, :], otk[:])
    nc.compile()
    return nc


def _toeplitz(kmat: np.ndarray) -> np.ndarray:
    """[128, KS*64] bf16: two identical [64, KS*64] stationary band-matrix
    strips (one per PE row-half). T[k, dj*64+m] = K[k-m, dj] for k-m in
    [0, KS)."""
    k_idx = np.arange(64)[:, None]
    m_idx = np.arange(64)[None, :]
    di = k_idx - m_idx
    mask = (di >= 0) & (di < KS)
    dic = np.clip(di, 0, KS - 1)
    t = np.zeros((64, KS, 64), dtype=np.float32)
    for dj in range(KS):
        t[:, dj, :] = np.where(mask, kmat[dic, dj], 0.0)
    t = t.reshape(64, KS * 64)
    return np.tile(t, (2, 1)).astype(ml_dtypes.bfloat16)


def _shard_inputs(image: np.ndarray, kmat: np.ndarray):
    tmat = _toeplitz(kmat)
    xb = image[:, C - 1].astype(ml_dtypes.bfloat16)  # [B, H, W]
    pad = np.zeros((B, HP, XW), dtype=ml_dtypes.bfloat16)
    pad[:, PAD : PAD + H, PAD : PAD + W] = xb

    p = np.arange(128)
    ki = (p >> 6)[:, None, None]                      # [128,1,1]
    q = (p & 63)[:, None, None]
    r = np.arange(ROUNDS)[None, :, None]              # [1,R,1]
    mj = np.arange(2)[None, None, :]                  # [1,1,2]
    g = 4 * r + 2 * mj + ki                           # [128,R,2] core-group id
    img_loc = g // NGI
    row = GR * (g % NGI) + q                          # [128,R,2]

    in_maps = []
    for i in range(NCORES):
        xi = pad[2 * i + img_loc, row, :]             # [128,R,2,XW]
        in_maps.append({"x": np.ascontiguousarray(xi), "tmat": tmat})
    return in_maps


def _unpack_output(results) -> np.ndarray:
    y = np.empty((B, H, W), dtype=np.float32)
    for i in range(NCORES):
        arr = np.asarray(results[i]["y"]).astype(np.float32)  # [128,R,2,W]
        for r in range(ROUNDS):
            for mj in range(2):
                for ki in range(2):
                    g = 4 * r + 2 * mj + ki
                    img = PER * i + g // NGI
                    r0 = GR * (g % NGI)
                    nv = min(GR, H - r0)
                    y[img, r0 : r0 + nv] = arr[64 * mj : 64 * mj + nv, r, ki]
    return y


def kernel(**inputs):
    global LAST_RESULTS
    image = np.asarray(inputs["image"], dtype=np.float32)
    kmat = np.asarray(inputs["kernel"], dtype=np.float32)
    assert image.shape == (B, C, H, W), image.shape

    if "nc" not in _CACHE:
        _CACHE["nc"] = _build_bass()
    nc = _CACHE["nc"]

    in_maps = _shard_inputs(image, kmat)
    res = run_bass_kernel_spmd(nc, in_maps, list(range(NCORES)))
    LAST_RESULTS = res

    y = _unpack_output(res.results)
    return np.broadcast_to(y[:, None], (B, C, H, W))


# revision 11
# speedup vs baseline: 1.4401x; 1.1326x over previous
"""Trainium2 Bass kernel for nn_Conv_8443905704574.

Reference semantics: 7x7 cross-correlation (stride 1, zero pad 3) applied to
the LAST input channel only; the single-channel result is broadcast to all 3
output channels.

Device algorithm: banded-Toeplitz matmul conv in bf16 using 64x64 PE-array
tiling. The 128x128 PE array is addressed as four 64x64 tiles
(tile_position=(64*ki, 64*mj)); the 4 matmuls of a tap-slot issue
back-to-back and execute concurrently on the quadrants (measured ~216 ns per
4-MM slot at N=512 vs 547 ns for a 16-MM 32x32 slot — fewer instructions and
a 1-wave XBUS schedule). Each tile convolves a 64-row window of the image
producing 58 valid output rows: the stationary is a [64,64] band matrix
(T[k,m] = K[k-m,dj]) per kernel column dj, the moving operand a W-shifted
[64,512] slice; 7 taps accumulate in fp32 PSUM. One round = 4 row-groups x
2 W-chunks = 14 slots; 9 rounds cover a core's 2 images.

DMA: every transfer spans all 128 SBUF partitions (the HWDGE sprays
descriptors across all 16 SDMA engines only for 128-partition transfers;
partial-partition stores collapse onto 2 engines at ~45 GB/s). Host packs
x[128, 9, 2, 1030] bf16 (partition 64*ki+q of (round r, mj) holds padded
image row 58*(4r+2mj+ki)+q) and unpacks y[128, 9, 2, 1024] bf16. PSUM banks
are drained by Scalar (ki=0) and Vector (ki=1) engines in parallel, casting
fp32->bf16; the last round stores per-ki so the final transfers overlap the
drain.

Sharding: pure data parallel - 2 images per core across 8 cores.
"""

import numpy as np
import ml_dtypes

import concourse.bacc as bacc
import concourse.mybir as mybir
import concourse.tile as tile
from concourse.bass_utils import run_bass_kernel_spmd

B, C, H, W = 16, 3, 1024, 1024
KS = 7
PAD = KS // 2
NCORES = 8
PER = B // NCORES            # images per core
GR = 64 - (KS - 1)           # 58 valid output rows per 64-row tile window
NGI = (H + GR - 1) // GR     # 18 row-groups per image
NG = PER * NGI               # 36 row-groups per core
ROUNDS = NG // 4             # 9 rounds of 4 concurrent tiles
XW = W + 2 * PAD             # host-padded input width (1030)
HP = GR * (NGI - 1) + 64     # host-padded input height (1050)

f32 = mybir.dt.float32
bf16 = mybir.dt.bfloat16

_CACHE = {}
LAST_RESULTS = None


def _build_bass():
    nc = bacc.Bacc("TRN2", target_bir_lowering=False, debug=False)
    x = nc.dram_tensor("x", [128, ROUNDS, 2, XW], bf16, kind="ExternalInput")
    tmat = nc.dram_tensor("tmat", [128, KS * 64], bf16, kind="ExternalInput")
    y = nc.dram_tensor("y", [128, ROUNDS, 2, W], bf16, kind="ExternalOutput")

    with tile.TileContext(nc) as tc:
        with (
            tc.tile_pool(name="xp", bufs=ROUNDS) as xpool,
            tc.tile_pool(name="tp", bufs=1) as tpool,
            tc.tile_pool(name="op", bufs=4) as opool,
            tc.tile_pool(name="pp", bufs=1, space="PSUM") as ppool,
            tc.tile_pool(name="wp", bufs=1) as wzpool,
        ):
            # 4 PSUM banks: (ki, chunk) -> one [128,512] bank holding the
            # 2 mj tiles' outputs stacked along partitions; +2 warmup banks.
            ps = [
                [
                    ppool.tile([128, 512], f32, name=f"ps{ki}{c}", tag=f"ps{ki}{c}")
                    for c in range(2)
                ]
                for ki in range(2)
            ]
            pz = [
                ppool.tile([128, 512], f32, name=f"pz{i}", tag=f"pz{i}")
                for i in range(2)
            ]

            # Stationaries first (tiny, gates the first real matmul), then
            # all input rounds up front - bufs=ROUNDS, so no reuse hazard
            # and the sync DGE FIFO never blocks on a compute dependency.
            ts = tpool.tile([128, KS * 64], bf16, name="ts")
            nc.sync.dma_start(ts[:], tmat[:])

            xgs = []
            for r in range(ROUNDS):
                xg = xpool.tile([128, 2 * XW], bf16, name=f"xg", tag="xg")
                xgs.append(xg)
                nc.sync.dma_start(xg[:], x[:, r, :, :])

            # PE warm-up: zero matmuls release the HAM clock gate so real
            # matmuls run at 2.4 GHz.
            wz = wzpool.tile([128, 128 + 512], bf16, name="wz")
            nc.vector.memset(wz[:], 0.0)
            for i in range(12):
                nc.tensor.matmul(
                    pz[i % 2][:],
                    wz[:, 0:128],
                    wz[:, 128 : 128 + 512],
                    start=True,
                    stop=True,
                )

            for r in range(ROUNDS):
                xg = xgs[r]
                for c in range(2):
                    for dj in range(KS):
                        for ki in range(2):
                            for mj in range(2):
                                nc.tensor.matmul(
                                    ps[ki][c][64 * mj : 64 * mj + 64, :],
                                    ts[64 * ki : 64 * ki + 64, dj * 64 : dj * 64 + 64],
                                    xg[
                                        64 * ki : 64 * ki + 64,
                                        mj * XW + c * 512 + dj : mj * XW + c * 512 + dj + 512,
                                    ],
                                    start=(dj == 0),
                                    stop=(dj == KS - 1),
                                    tile_position=(64 * ki, 64 * mj),
                                )
                if r < ROUNDS - 1:
                    ot = opool.tile([128, 2 * W], bf16, name="ot", tag="ot")
                    for ki in range(2):
                        for c in range(2):
                            dst = ot[:, ki * W + c * 512 : ki * W + c * 512 + 512]
                            if ki == 0:
                                nc.scalar.copy(dst, ps[ki][c][:])
                            else:
                                nc.vector.tensor_copy(dst, ps[ki][c][:])
                    nc.sync.dma_start(y[:, r, :, :], ot[:])
                else:
                    # Last round: store per ki as soon as its two banks are
                    # drained, so the final transfers overlap the drain.
                    for ki in range(2):
                        otk = opool.tile(
                            [128, W], bf16, name=f"otl{ki}", tag=f"otl{ki}"
                        )
                        for c in range(2):
                            dst = otk[:, c * 512 : c * 512 + 512]
                            if ki == 0:
                                nc.scalar.copy(dst, ps[ki][c][:])
                            else:
                                nc.vector.tensor_copy(dst, ps[ki][c][:])
                        nc.sync.dma_start(y[:, r, ki, :], otk[:])
    nc.compile()
    return nc


def _toeplitz(kmat: np.ndarray) -> np.ndarray:
    """[128, KS*64] bf16: two identical [64, KS*64] stationary band-matrix
    strips (one per PE row-half). T[k, dj*64+m] = K[k-m, dj] for k-m in
    [0, KS)."""
    k_idx = np.arange(64)[:, None]
    m_idx = np.arange(64)[None, :]
    di = k_idx - m_idx
    mask = (di >= 0) & (di < KS)
    dic = np.clip(di, 0, KS - 1)
    t = np.zeros((64, KS, 64), dtype=np.float32)
    for dj in range(KS):
        t[:, dj, :] = np.where(mask, kmat[dic, dj], 0.0)
    t = t.reshape(64, KS * 64)
    return np.tile(t, (2, 1)).astype(ml_dtypes.bfloat16)


def _shard_inputs(image: np.ndarray, kmat: np.ndarray):
    tmat = _toeplitz(kmat)
    xb = image[:, C - 1].astype(ml_dtypes.bfloat16)  # [B, H, W]
    pad = np.zeros((B, HP, XW), dtype=ml_dtypes.bfloat16)
    pad[:, PAD : PAD + H, PAD : PAD + W] = xb

    p = np.arange(128)
    ki = (p >> 6)[:, None, None]                      # [128,1,1]
    q = (p & 63)[:, None, None]
    r = np.arange(ROUNDS)[None, :, None]              # [1,R,1]
    mj = np.arange(2)[None, None, :]                  # [1,1,2]
    g = 4 * r + 2 * mj + ki                           # [128,R,2] core-group id
    img_loc = g // NGI
    row = GR * (g % NGI) + q                          # [128,R,2]

    in_maps = []
    for i in range(NCORES):
        xi = pad[2 * i + img_loc, row, :]             # [128,R,2,XW]
        in_maps.append({"x": np.ascontiguousarray(xi), "tmat": tmat})
    return in_maps


def _unpack_output(results) -> np.ndarray:
    y = np.empty((B, H, W), dtype=np.float32)
    for i in range(NCORES):
        arr = np.asarray(results[i]["y"]).astype(np.float32)  # [128,R,2,W]
        for r in range(ROUNDS):
            for mj in range(2):
                for ki in range(2):
                    g = 4 * r + 2 * mj + ki
                    img = PER * i + g // NGI
                    r0 = GR * (g % NGI)
                    nv = min(GR, H - r0)
                    y[img, r0 : r0 + nv] = arr[64 * mj : 64 * mj + nv, r, ki]
    return y


def kernel(**inputs):
    global LAST_RESULTS
    image = np.asarray(inputs["image"], dtype=np.float32)
    kmat = np.asarray(inputs["kernel"], dtype=np.float32)
    assert image.shape == (B, C, H, W), image.shape

    if "nc" not in _CACHE:
        _CACHE["nc"] = _build_bass()
    nc = _CACHE["nc"]

    in_maps = _shard_inputs(image, kmat)
    res = run_bass_kernel_spmd(nc, in_maps, list(range(NCORES)))
    LAST_RESULTS = res

    y = _unpack_output(res.results)
    return np.broadcast_to(y[:, None], (B, C, H, W))


# revision 12
# speedup vs baseline: 1.4683x; 1.0195x over previous
"""Trainium2 Bass kernel for nn_Conv_8443905704574.

Reference semantics: 7x7 cross-correlation (stride 1, zero pad 3) applied to
the LAST input channel only; the single-channel result is broadcast to all 3
output channels.

Device algorithm: banded-Toeplitz matmul conv in bf16 using 64x64 PE-array
tiling. The 128x128 PE array is addressed as four 64x64 tiles
(tile_position=(64*ki, 64*mj)); the 4 matmuls of a tap-slot issue
back-to-back and execute concurrently on the quadrants (measured ~216 ns per
4-MM slot at N=512 vs 547 ns for a 16-MM 32x32 slot — fewer instructions and
a 1-wave XBUS schedule). Each tile convolves a 64-row window of the image
producing 58 valid output rows: the stationary is a [64,64] band matrix
(T[k,m] = K[k-m,dj]) per kernel column dj, the moving operand a W-shifted
[64,512] slice; 7 taps accumulate in fp32 PSUM. One round = 4 row-groups x
2 W-chunks = 14 slots; 9 rounds cover a core's 2 images.

DMA: every transfer spans all 128 SBUF partitions (the HWDGE sprays
descriptors across all 16 SDMA engines only for 128-partition transfers;
partial-partition stores collapse onto 2 engines at ~45 GB/s). Host packs
x[128, 9, 2, 1030] bf16 (partition 64*ki+q of (round r, mj) holds padded
image row 58*(4r+2mj+ki)+q) and unpacks y[128, 9, 2, 1024] bf16. PSUM banks
are drained by Scalar (ki=0) and Vector (ki=1) engines in parallel, casting
fp32->bf16; the last round stores per-ki so the final transfers overlap the
drain.

Sharding: pure data parallel - 2 images per core across 8 cores.
"""

import numpy as np
import ml_dtypes

import concourse.bacc as bacc
import concourse.mybir as mybir
import concourse.tile as tile
from concourse.bass_utils import run_bass_kernel_spmd

B, C, H, W = 16, 3, 1024, 1024
KS = 7
PAD = KS // 2
NCORES = 8
PER = B // NCORES            # images per core
GR = 64 - (KS - 1)           # 58 valid output rows per 64-row tile window
NGI = (H + GR - 1) // GR     # 18 row-groups per image
NG = PER * NGI               # 36 row-groups per core
ROUNDS = NG // 4             # 9 rounds of 4 concurrent tiles
XW = W + 2 * PAD             # host-padded input width (1030)
HP = GR * (NGI - 1) + 64     # host-padded input height (1050)

f32 = mybir.dt.float32
bf16 = mybir.dt.bfloat16

_CACHE = {}
LAST_RESULTS = None


def _build_bass():
    nc = bacc.Bacc("TRN2", target_bir_lowering=False, debug=False)
    x = nc.dram_tensor("x", [128, ROUNDS, 2, XW], bf16, kind="ExternalInput")
    tmat = nc.dram_tensor("tmat", [128, KS * 64], bf16, kind="ExternalInput")
    y = nc.dram_tensor("y", [128, ROUNDS, 2, W], bf16, kind="ExternalOutput")

    with tile.TileContext(nc) as tc:
        with (
            tc.tile_pool(name="xp", bufs=ROUNDS) as xpool,
            tc.tile_pool(name="tp", bufs=1) as tpool,
            tc.tile_pool(name="op", bufs=4) as opool,
            tc.tile_pool(name="pp", bufs=1, space="PSUM") as ppool,
            tc.tile_pool(name="wp", bufs=1) as wzpool,
        ):
            # 4 PSUM banks: (ki, chunk) -> one [128,512] bank holding the
            # 2 mj tiles' outputs stacked along partitions; +2 warmup banks.
            ps = [
                [
                    ppool.tile([128, 512], f32, name=f"ps{ki}{c}", tag=f"ps{ki}{c}")
                    for c in range(2)
                ]
                for ki in range(2)
            ]
            pz = [
                ppool.tile([128, 512], f32, name=f"pz{i}", tag=f"pz{i}")
                for i in range(2)
            ]

            # Stationaries first (tiny, gates the first real matmul), then
            # all input rounds up front - bufs=ROUNDS, so no reuse hazard
            # and the sync DGE FIFO never blocks on a compute dependency.
            ts = tpool.tile([128, KS * 64], bf16, name="ts")
            nc.sync.dma_start(ts[:], tmat[:])

            xgs = []
            for r in range(ROUNDS):
                xg = xpool.tile([128, 2 * XW], bf16, name=f"xg", tag="xg")
                xgs.append(xg)
                nc.sync.dma_start(xg[:], x[:, r, :, :])

            # PE warm-up: zero matmuls release the HAM clock gate so real
            # matmuls run at 2.4 GHz.
            wz = wzpool.tile([128, 128 + 512], bf16, name="wz")
            nc.vector.memset(wz[:], 0.0)
            for i in range(9):
                nc.tensor.matmul(
                    pz[i % 2][:],
                    wz[:, 0:128],
                    wz[:, 128 : 128 + 512],
                    start=True,
                    stop=True,
                )

            for r in range(ROUNDS):
                xg = xgs[r]
                for c in range(2):
                    for dj in range(KS):
                        for ki in range(2):
                            for mj in range(2):
                                nc.tensor.matmul(
                                    ps[ki][c][64 * mj : 64 * mj + 64, :],
                                    ts[64 * ki : 64 * ki + 64, dj * 64 : dj * 64 + 64],
                                    xg[
                                        64 * ki : 64 * ki + 64,
                                        mj * XW + c * 512 + dj : mj * XW + c * 512 + dj + 512,
                                    ],
                                    start=(dj == 0),
                                    stop=(dj == KS - 1),
                                    tile_position=(64 * ki, 64 * mj),
                                )
                if r < ROUNDS - 1:
                    ot = opool.tile([128, 2 * W], bf16, name="ot", tag="ot")
                    for ki in range(2):
                        for c in range(2):
                            dst = ot[:, ki * W + c * 512 : ki * W + c * 512 + 512]
                            if ki == 0:
                                nc.scalar.copy(dst, ps[ki][c][:])
                            else:
                                nc.vector.tensor_copy(dst, ps[ki][c][:])
                    nc.sync.dma_start(y[:, r, :, :], ot[:])
                else:
                    # Last round: store per ki as soon as its two banks are
                    # drained, so the final transfers overlap the drain.
                    for ki in range(2):
                        otk = opool.tile(
                            [128, W], bf16, name=f"otl{ki}", tag=f"otl{ki}"
                        )
                        for c in range(2):
                            dst = otk[:, c * 512 : c * 512 + 512]
                            if ki == 0:
                                nc.scalar.copy(dst, ps[ki][c][:])
                            else:
                                nc.vector.tensor_copy(dst, ps[ki][c][:])
                        nc.sync.dma_start(y[:, r, ki, :], otk[:])
    nc.compile()
    return nc


def _toeplitz(kmat: np.ndarray) -> np.ndarray:
    """[128, KS*64] bf16: two identical [64, KS*64] stationary band-matrix
    strips (one per PE row-half). T[k, dj*64+m] = K[k-m, dj] for k-m in
    [0, KS)."""
    k_idx = np.arange(64)[:, None]
    m_idx = np.arange(64)[None, :]
    di = k_idx - m_idx
    mask = (di >= 0) & (di < KS)
    dic = np.clip(di, 0, KS - 1)
    t = np.zeros((64, KS, 64), dtype=np.float32)
    for dj in range(KS):
        t[:, dj, :] = np.where(mask, kmat[dic, dj], 0.0)
    t = t.reshape(64, KS * 64)
    return np.tile(t, (2, 1)).astype(ml_dtypes.bfloat16)


def _shard_inputs(image: np.ndarray, kmat: np.ndarray):
    tmat = _toeplitz(kmat)
    xb = image[:, C - 1].astype(ml_dtypes.bfloat16)  # [B, H, W]
    pad = np.zeros((B, HP, XW), dtype=ml_dtypes.bfloat16)
    pad[:, PAD : PAD + H, PAD : PAD + W] = xb

    p = np.arange(128)
    ki = (p >> 6)[:, None, None]                      # [128,1,1]
    q = (p & 63)[:, None, None]
    r = np.arange(ROUNDS)[None, :, None]              # [1,R,1]
    mj = np.arange(2)[None, None, :]                  # [1,1,2]
    g = 4 * r + 2 * mj + ki                           # [128,R,2] core-group id
    img_loc = g // NGI
    row = GR * (g % NGI) + q                          # [128,R,2]

    in_maps = []
    for i in range(NCORES):
        xi = pad[2 * i + img_loc, row, :]             # [128,R,2,XW]
        in_maps.append({"x": np.ascontiguousarray(xi), "tmat": tmat})
    return in_maps


def _unpack_output(results) -> np.ndarray:
    y = np.empty((B, H, W), dtype=np.float32)
    for i in range(NCORES):
        arr = np.asarray(results[i]["y"]).astype(np.float32)  # [128,R,2,W]
        for r in range(ROUNDS):
            for mj in range(2):
                for ki in range(2):
                    g = 4 * r + 2 * mj + ki
                    img = PER * i + g // NGI
                    r0 = GR * (g % NGI)
                    nv = min(GR, H - r0)
                    y[img, r0 : r0 + nv] = arr[64 * mj : 64 * mj + nv, r, ki]
    return y


def kernel(**inputs):
    global LAST_RESULTS
    image = np.asarray(inputs["image"], dtype=np.float32)
    kmat = np.asarray(inputs["kernel"], dtype=np.float32)
    assert image.shape == (B, C, H, W), image.shape

    if "nc" not in _CACHE:
        _CACHE["nc"] = _build_bass()
    nc = _CACHE["nc"]

    in_maps = _shard_inputs(image, kmat)
    res = run_bass_kernel_spmd(nc, in_maps, list(range(NCORES)))
    LAST_RESULTS = res

    y = _unpack_output(res.results)
    return np.broadcast_to(y[:, None], (B, C, H, W))
